# revision 19
# baseline (speedup 1.0000x reference)
"""Distributed causal attention for TRN2 (8 NeuronCores).

Reference op: qkv = x @ w_qkv (+0 bias); per-head causal softmax(q k^T/8) v
(16 heads, hd=64); concat; out = . @ w_proj (+0 bias).

Sharding: head-parallel attention (2 heads/core) with an AllToAll
redistribution to sequence-parallel for the output projection (each core owns
S/8 = 512 query rows). Two 256-wide query windows per block, one AllToAll per
window.

v5 structure — a single globally software-pipelined stream:
  * Work items (window, qblock) are interleaved qb-major (ORDER) so scalar
    (exp) and tensor load stay uniform; window 0 completes at position 10/16
    so its AllToAll overlaps the remaining window-1 compute.
  * Per item, groups of 2 sk-tiles x 2 heads: QK (row-split across the two
    heads' PE row-groups) -> one [128,1024] exp on ACT -> PV trailing one
    group. The last group's PV + normalize are CARRIED into the next item
    after its first QK so item boundaries never serialize exp->PV->QK->exp.
  * QKV projection is chopped into half-K filler closures with causal
    deadlines (window-0 of block n+1 needs block n's second-half k/v), popped
    between groups; output projections run in the tail, covering the final
    AllToAll's skew window, topped up with HAM-warming dummy matmuls.
  * x/w/wproj are uploaded in block-major host layouts (8KB DMA lines); x
    chunks split across the sync+scalar HWDGE queues.
  * Softmax denominators ride as a ones-column in V (PV lhsT = [v_h | 1],
    M=65); normalization broadcasts recip(den) via a DRAM round-trip
    (partition-stride-0 read), except the last item which uses a K=1 matmul
    broadcast to shorten the a2a-feeding chain.

All matmuls bf16 (fp32 PSUM); softmax without max-subtraction (|score| < 4).
kernel(**inputs) takes FULL fp32 inputs, returns the FULL fp32 output.
"""

from collections import deque

import numpy as np
import ml_dtypes

import concourse.bacc as bacc
import concourse.bass as bass
import concourse.tile as tile
from concourse import mybir
from concourse.bass_utils import run_bass_kernel_spmd

N_CORES = 8
D = 1024
H = 16
HD = 64
HPC = H // N_CORES          # heads per core = 2
MQKV = 3 * HPC * HD         # per-core qkv feature cols = 384

BF16 = mybir.dt.bfloat16
F32 = mybir.dt.float32
bf16 = ml_dtypes.bfloat16

# Bumping this changes the compiled executable's signature (a dummy input's
# shape encodes it), forcing a fresh compile + stage.
BUILD_SALT = 27


def build(S):
    QB = S // N_CORES        # query rows per core (A2A shard) = 512
    NQ = N_CORES
    NB = S // 512            # seq blocks of 512 = 8
    QW = 256
    WINDOWS = [(0, QW), (QW, QW)]
    # window 0 finishes at position 10 so its A2A + output projection have
    # ~80us of window-1 work to hide behind (robust to collective skew).
    ORDER = [(0, 0), (0, 1), (0, 2), (1, 0), (0, 3), (1, 1), (0, 4), (1, 2),
             (0, 5), (0, 6), (0, 7), (1, 3), (1, 4), (1, 5), (1, 6), (1, 7)]
    POSI = {it: p for p, it in enumerate(ORDER)}

    nc = bacc.Bacc("TRN2", num_devices=N_CORES)

    # block-major x: xB[p, n, a, s] = x[a*128+p, n*512+s]  (contig 8KB/line)
    xB = nc.declare_dram_parameter("xB", [128, NB, 8, 512], BF16, isOutput=False)
    wqkv = nc.declare_dram_parameter("wqkv", [128, 8, MQKV], BF16, isOutput=False)
    wproj = nc.declare_dram_parameter("wproj", [128, 8, D], BF16, isOutput=False)
    maskp = nc.declare_dram_parameter("mask", [128, 2 * 128], BF16, isOutput=False)
    salt = nc.declare_dram_parameter("salt", [1, BUILD_SALT], F32, isOutput=False)
    out_ext = nc.declare_dram_parameter("out", [QB, D], F32, isOutput=True)

    a2a_in = [nc.dram_tensor(f"a2a_in{iw}", [NQ, 2 * HD, qw], BF16)
              for iw, (q0, qw) in enumerate(WINDOWS)]
    a2a_out = [nc.dram_tensor(f"a2a_out{iw}", [NQ, 2 * HD, qw], BF16)
               for iw, (q0, qw) in enumerate(WINDOWS)]
    rden_dram = nc.dram_tensor("rden_dram", [HPC, NQ, QB], F32)
    warm_in = nc.dram_tensor("warm_in", [NQ, 1, 64], BF16)
    warm_out = nc.dram_tensor("warm_out", [NQ, 1, 64], BF16)

    with tile.TileContext(nc) as tc:
        with (
            tc.tile_pool(name="singles", bufs=1) as singles,
            tc.tile_pool(name="work", bufs=2) as work,
            tc.tile_pool(name="norm", bufs=4) as norm,
            tc.tile_pool(name="ppool", bufs=4) as ppool,
            tc.tile_pool(name="upool", bufs=4) as upool,
            tc.tile_pool(name="psq", bufs=2, space="PSUM") as psq,
            tc.tile_pool(name="pso", bufs=2, space="PSUM") as pso,
            tc.tile_pool(name="psm", bufs=2, space="PSUM") as psm,
        ):
            x_sb = singles.tile([128, NB, 8, 512], BF16)
            nc.sync.dma_start(out=x_sb[:, 0, 0:4], in_=xB[:, 0, 0:4])
            nc.scalar.dma_start(out=x_sb[:, 0, 4:8], in_=xB[:, 0, 4:8])
            w_sb = singles.tile([128, 8, MQKV], BF16)
            nc.sync.dma_start(out=w_sb[:], in_=wqkv[:])
            # warmup AllToAll: staged early so the collective path is warm +
            # entry skew partly absorbed before a2a(0).
            warm_sb = singles.tile([NQ, 64], BF16)
            nc.vector.memset(warm_sb[:], 0.0)
            nc.sync.dma_start(out=warm_in[:, 0, :], in_=warm_sb[:])
            nc.gpsimd.collective_compute(
                "AllToAll", mybir.AluOpType.bypass,
                replica_groups=[list(range(N_CORES))],
                ins=[warm_in[:]], outs=[warm_out[:]],
            )

            # remaining x chunks split across the two HWDGE queues
            # (sync + scalar) so the stream keeps ahead of the consumers.
            mask_sb = singles.tile([128, 256], BF16)
            nc.sync.dma_start(out=mask_sb[:], in_=maskp[:])
            salt_sb = singles.tile([1, BUILD_SALT], F32)
            nc.sync.dma_start(out=salt_sb[:], in_=salt[:])
            for n in range(1, NB):
                nc.sync.dma_start(out=x_sb[:, n, 0:4], in_=xB[:, n, 0:4])
                nc.scalar.dma_start(out=x_sb[:, n, 4:8], in_=xB[:, n, 4:8])
            wp_sb = singles.tile([128, 8, D], BF16)   # DMA deferred to pos 6

            ones_sb = singles.tile([128, HD], F32)
            nc.vector.memset(ones_sb[:], 1.0)
            qkvT = singles.tile([128, 2, S], BF16)
            VST = 160
            v_sb = singles.tile([128, S // 128, VST], BF16)
            nc.vector.memset(v_sb[:, :, HD:HD + 1], 1.0)
            nc.vector.memset(v_sb[:, :, 80 + HD:80 + HD + 1], 1.0)

            # ---- self-contained filler closures, deadline-ordered ----
            def emit_proj_qk(n, m, nh):
                ps = psm.tile([128, 256], F32, name=f"psqk{n}_{m}_{nh}", tag="psm")
                for a in range(8):
                    nc.tensor.matmul(
                        ps[:],
                        lhsT=w_sb[:, a, 128 * m:128 * (m + 1)],
                        rhs=x_sb[:, n, a, 256 * nh:256 * (nh + 1)],
                        start=(a == 0), stop=(a == 7),
                    )
                nc.vector.tensor_copy(
                    qkvT[:, m, 512 * n + 256 * nh:512 * n + 256 * (nh + 1)], ps[:])

            def emit_proj_v(n, t):
                psv = psm.tile([128, 256], F32, name=f"psv{t}", tag="psm")[:, 0:128]
                for a in range(8):
                    nc.tensor.matmul(
                        psv[:],
                        lhsT=x_sb[:, n, a, 128 * (t - 4 * n):128 * (t - 4 * n + 1)],
                        rhs=w_sb[:, a, 256:384],
                        start=(a == 0), stop=(a == 7),
                    )
                nc.vector.tensor_copy(v_sb[:, t, 0:HD], psv[:, 0:HD])
                nc.vector.tensor_copy(v_sb[:, t, 80:80 + HD], psv[:, HD:2 * HD])

            fillers = []   # (deadline, min_pos, seq, closure)
            seq = [0]

            def add_filler(dl, mp, fn):
                fillers.append((dl, mp, seq[0], fn))
                seq[0] += 1

            for n in range(NB):
                p0, p1 = POSI[(0, n)], POSI[(1, n)]
                # k/v second halves (tiles 4n+2, 4n+3) are consumed causally
                # by window-0 of block n+1 as well as window-1 of block n —
                # deadline is whichever comes first in the stream.
                pk = min(POSI.get((0, n + 1), 10 ** 6), p1)
                add_filler(p0, -1, (lambda n=n: emit_proj_qk(n, 0, 0)))
                add_filler(p0, -1, (lambda n=n: emit_proj_qk(n, 1, 0)))
                add_filler(p1, -1, (lambda n=n: emit_proj_qk(n, 0, 1)))
                add_filler(pk, -1, (lambda n=n: emit_proj_qk(n, 1, 1)))
                add_filler(p0 + 0.3, -1, (lambda n=n: emit_proj_v(n, 4 * n)))
                add_filler(p0 + 0.3, -1, (lambda n=n: emit_proj_v(n, 4 * n + 1)))
                add_filler(pk, -1, (lambda n=n: emit_proj_v(n, 4 * n + 2)))
                add_filler(pk, -1, (lambda n=n: emit_proj_v(n, 4 * n + 3)))
            fillers.sort(key=lambda f: (f[0], f[2]))
            fillers = deque(fillers)

            ao_tiles = {}
            ob_tiles = {}

            def load_ao(iw):
                (q0, qw) = WINDOWS[iw]
                ao = singles.tile([128, NQ, qw], BF16, name=f"ao{iw}", tag=f"ao{iw}")
                ao_tiles[iw] = ao
                for g in range(NQ):
                    nc.sync.dma_start(out=ao[:, g, :], in_=a2a_out[iw][g])

            def emit_outproj_ob(iw, mt):
                ob_tiles[(iw, mt)] = work.tile(
                    [128, D], F32, name=f"ob{iw}_{mt}", tag="ob")

            def emit_outproj_mm(iw, mt, nh):
                (q0, qw) = WINDOWS[iw]
                pf = psm.tile([128, 512], F32, name=f"pf{iw}_{mt}_{nh}", tag="psm")
                ao = ao_tiles[iw]
                mo = 128 * mt
                for g in range(8):
                    nc.tensor.matmul(
                        pf[:],
                        lhsT=ao[:, g, mo:mo + 128],
                        rhs=wp_sb[:, g, 512 * nh:512 * (nh + 1)],
                        start=(g == 0), stop=(g == 7),
                    )
                ob = ob_tiles[(iw, mt)]
                nc.vector.tensor_copy(ob[:, 512 * nh:512 * (nh + 1)], pf[:])
                eng = nc.sync if nh == 0 else nc.scalar
                eng.dma_start(
                    out=out_ext[q0 + mo:q0 + mo + 128, 512 * nh:512 * (nh + 1)],
                    in_=ob[:, 512 * nh:512 * (nh + 1)])

            def a2a(iw):
                nc.gpsimd.collective_compute(
                    "AllToAll", mybir.AluOpType.bypass,
                    replica_groups=[list(range(N_CORES))],
                    ins=[a2a_in[iw][:]], outs=[a2a_out[iw][:]],
                )

            def pop_fillers(pos_frac, budget=1):
                # all past-deadline closures, then up to `budget` extras
                n = 0
                while fillers:
                    dl, mp, _, fn = fillers[0]
                    if dl <= pos_frac or (n < budget and mp <= pos_frac
                                          and dl <= pos_frac + 1.0):
                        fillers.popleft()
                        fn()
                        n += 1
                    else:
                        break

            # ---- the main pipelined stream ----
            for pos, (iw, qb) in enumerate(ORDER):
                if pos == 6:
                    nc.sync.dma_start(out=wp_sb[:], in_=wproj[:])
                pop_fillers(pos, budget=0)
                (q0, qw) = WINDOWS[iw]
                nk = 4 * qb + (q0 + qw) // 128
                ng = nk // 2
                qlo = QB * qb + q0

                po = [pso.tile([HD + 1, 512], F32, name=f"po{pos}_{h}",
                               tag="pso")[:, 0:256] for h in range(HPC)]
                pts = [None] * ng

                def emit_pv(g, h):
                    for j in range(2):
                        t = 2 * g + j
                        nc.tensor.matmul(
                            po[h][:],
                            lhsT=v_sb[:, t, 80 * h:80 * h + HD + 1],
                            rhs=pts[g][:, 512 * h + 256 * j:512 * h + 256 * (j + 1)],
                            start=(t == 0), stop=(t == nk - 1),
                        )

                def normalize(h):
                    un = upool.tile([HD + 1, 256], F32, name=f"un{pos}_{h}",
                                    tag="unorm")
                    nc.vector.tensor_copy(un[:], po[h][:])
                    nc.vector.reciprocal(un[HD:HD + 1, :], un[HD:HD + 1, :])
                    if pos == len(ORDER) - 1:
                        bcp = psm.tile([HD, 512], F32, name=f"bcp{h}",
                                       tag="psm")[:, 0:256]
                        nc.tensor.matmul(
                            bcp[:], lhsT=ones_sb[HD:HD + 1, :],
                            rhs=un[HD:HD + 1, :], start=True, stop=True)
                        st = norm.tile([HD, 256], BF16, name=f"stz{h}",
                                       tag="stage")
                        nc.vector.tensor_mul(st[:], un[0:HD, :], bcp[:])
                        nc.sync.dma_start(
                            out=a2a_in[iw][qb, HD * h:HD * (h + 1), :],
                            in_=st[:])
                        return
                    nc.sync.dma_start(
                        out=rden_dram[h, qb, q0:q0 + qw], in_=un[HD:HD + 1, :])
                    bc = norm.tile([HD, 256], F32, name=f"bc{pos}_{h}", tag="bcast")
                    src = bass.AP(
                        tensor=rden_dram,
                        offset=(h * NQ + qb) * QB + q0,
                        ap=[[0, HD], [1, qw]],
                    )
                    nc.sync.dma_start(out=bc[:], in_=src)
                    st = norm.tile([HD, 256], BF16, name=f"st{pos}_{h}", tag="stage")
                    nc.vector.tensor_mul(st[:], un[0:HD, :], bc[:])
                    nc.sync.dma_start(
                        out=a2a_in[iw][qb, HD * h:HD * (h + 1), :], in_=st[:])

                for g in range(ng):
                    ps = psq.tile([128, 1024], F32, name=f"ps{pos}_{g}", tag="psq")
                    for j in range(2):
                        t = 2 * g + j
                        for h in range(HPC):
                            nc.tensor.matmul(
                                ps[:, 512 * h + 256 * j:512 * h + 256 * (j + 1)],
                                lhsT=qkvT[HD * h:HD * (h + 1), 1, 128 * t:128 * (t + 1)],
                                rhs=qkvT[HD * h:HD * (h + 1), 0, qlo:qlo + qw],
                                start=True, stop=True,
                            )
                    pt = ppool.tile([128, 1024], BF16, name=f"pt{pos}_{g}", tag="p")
                    nc.scalar.activation(
                        pt[:], ps[:], mybir.ActivationFunctionType.Exp, scale=0.125)
                    pts[g] = pt
                    if g == ng - 1:
                        for h in range(HPC):
                            nc.vector.tensor_mul(
                                pt[:, 512 * h:512 * h + 128],
                                pt[:, 512 * h:512 * h + 128],
                                mask_sb[:, 128:256])
                            nc.vector.tensor_mul(
                                pt[:, 512 * h + 256:512 * h + 512],
                                pt[:, 512 * h + 256:512 * h + 512],
                                mask_sb[:, 0:256])
                    if g >= 1:
                        for h in range(HPC):
                            emit_pv(g - 1, h)
                    pop_fillers(pos + (g + 1) / ng, budget=1)
                # last group per head: PV then its normalize immediately, so
                # the a2a-feeding chain starts as early as possible
                for h in range(HPC):
                    emit_pv(ng - 1, h)
                    normalize(h)

                if (iw, qb) == (0, NB - 1):
                    a2a(0)
                    load_ao(0)

            # ---- tail ----
            # window-0's output projection runs here, AFTER the a2a(1)
            # trigger: it is ~17us of real PE work that hides the a2a(1)
            # skew/transfer window (instead of burning dummies), and it can
            # no longer head-of-line block the stream if a2a(0) runs long.
            pop_fillers(10 ** 9, budget=100)
            a2a(1)
            load_ao(1)
            for mt in range(2):
                emit_outproj_ob(0, mt)
                for nh in range(2):
                    emit_outproj_mm(0, mt, nh)
            # short dummy-matmul chain keeps HAM warm across any residual
            # a2a(1) wait (ends in a read so DCE can't drop it)
            pd = psm.tile([128, 512], F32, name="pd", tag="psm")
            NDUM = 90
            ao0 = ao_tiles[0]
            for i in range(NDUM):
                nc.tensor.matmul(
                    pd[:], lhsT=ao0[:, i % 8, 0:128], rhs=wp_sb[:, i % 8, 0:512],
                    start=(i == 0), stop=(i == NDUM - 1),
                )
            dsink = work.tile([1, 1], F32, name="dsink", tag="dsink")
            nc.vector.tensor_copy(dsink[:], pd[0:1, 0:1])
            nc.sync.dma_start(out=rden_dram[0, 0, 0:1], in_=dsink[:])
            for mt in range(2):
                emit_outproj_ob(1, mt)
                for nh in range(2):
                    emit_outproj_mm(1, mt, nh)

    nc.compile()
    return nc


def make_in_maps(S, x, w_qkv, b_qkv, w_proj, b_proj):
    """Host-side sharding: returns per-core input dicts (bf16-cast)."""
    NB = S // 512
    x2 = np.ascontiguousarray(x.reshape(S, D))
    xT = np.ascontiguousarray(x2.T).astype(bf16)          # [1024, S]
    # block-major upload layout: [128p, NB, 8a, 512s]
    xB = np.ascontiguousarray(
        xT.reshape(8, 128, NB, 512).transpose(1, 2, 0, 3)
    )
    wproj_b = np.ascontiguousarray(
        w_proj.astype(bf16).reshape(8, 128, D).transpose(1, 0, 2)
    )
    i, j = np.indices((128, 128))
    tri = (i <= j).astype(bf16)
    mask = np.concatenate([np.zeros((128, 128), bf16), tri], axis=1)
    in_maps = []
    for c in range(N_CORES):
        cols = []
        for part in range(3):  # q, k, v
            for hh in range(HPC):
                h = HPC * c + hh
                lo = part * D + HD * h
                cols.append(w_qkv[:, lo:lo + HD])
        w_c = np.concatenate(cols, axis=1).astype(bf16)   # [1024, 384]
        w_cb = np.ascontiguousarray(w_c.reshape(8, 128, MQKV).transpose(1, 0, 2))
        in_maps.append({
            "xB": xB,
            "wqkv": w_cb,
            "wproj": wproj_b,
            "mask": np.ascontiguousarray(mask),
            "salt": np.zeros((1, BUILD_SALT), np.float32),
        })
    return in_maps


_CACHE = {}


def _get_nc(S):
    if S not in _CACHE:
        _CACHE[S] = build(S)
    return _CACHE[S]


def kernel(x, w_qkv, b_qkv, w_proj, b_proj, trace=False):
    x = np.asarray(x, dtype=np.float32)
    w_qkv = np.asarray(w_qkv, dtype=np.float32)
    b_qkv = np.asarray(b_qkv, dtype=np.float32)
    w_proj = np.asarray(w_proj, dtype=np.float32)
    b_proj = np.asarray(b_proj, dtype=np.float32)
    B, S, _ = x.shape
    nc = _get_nc(S)
    in_maps = make_in_maps(S, x, w_qkv, b_qkv, w_proj, b_proj)
    res = run_bass_kernel_spmd(nc, in_maps, core_ids=list(range(N_CORES)), trace=trace)
    QB = S // N_CORES
    out = np.empty((S, D), dtype=np.float32)
    for c in range(N_CORES):
        out[QB * c:QB * (c + 1)] = res.results[c]["out"]
    if trace:
        kernel.last_exec_time_ns = res.exec_time_ns
        kernel.last_result = res
    return out.reshape(B, S, D)
